# revision 39
# baseline (speedup 1.0000x reference)
"""MoE-ALU (add with carry + xor over one-hot byte encodings) on 8 NeuronCores.

Semantics (validated against the jax reference bit-exactly): inputs a, b are
exact one-hot byte encodings [B, 4, 256] (little-endian bytes of 32-bit ints);
with SCALE=100 every softmax in the reference collapses to an exact one-hot, so

    out[0] = one_hot bytes of (a_int + b_int) mod 2^32
    out[1] = one_hot bytes of (a_int ^ b_int)

Layout: the host stores the one-hot inputs group/partition-major as fp8
([group, partition, chunk*column]; 0.0/1.0 are exact in fp8e4) so every DMA
descriptor is one contiguous 4 KiB run per partition, and the outputs as bf16
one-hots (exact 0/1). The device moves 8 MiB in + 16 MiB out per core instead
of 32+32 for f32 batch-major. All compute happens on device; the host only
reorders/recodes losslessly.

Device pipeline per 512-row batch group (8 groups per core):
  decode  TensorE: 16 accumulating matmuls (K=128 chunk each) of the fp8
          one-hot slabs against bf16 iota/256*iota weight columns produce
          PSUM [6, 512] = (a_lo16, a_hi16, b_lo16, b_hi16, s_lo_raw,
          s_hi_raw) -- the raw half sums come free from the PE (cost is
          N-only), exact in f32.
  stage   ScalarE copies PSUM -> SBUF f32 (frees the bank for group g+2).
  flip    TensorE transposes [6, 128] -> PSUM [128, 6] per 128-row tile.
  alu     VectorE per tile (6 ops): int32 cast, halves xor, carry fold,
          fused shift+mask byte extract, f32 cast; all 4 tiles of a group
          run phase-interleaved so every RAW wait's producer is >=4 ops
          back (the DVE pipe does not self-interlock; adjacent RAW stalls
          ~230ns).
  encode  per output byte, one-hot = is_equal against an iota table with a
          per-partition scalar: 6 bytes as DVE tensor_scalar (bf16 4x perf
          mode), 2 bytes as ScalarE Square/Relu pairs --
          relu(1-(idx-iota)^2), interleaved with their own sems.
  store   GPSIMD issues two 256 KiB output DMAs per tile as soon as that
          tile's bytes are encoded.

Raw Bass (one sync wait per instruction); rotating per-slot semaphores gate
buffer reuse; DVE same-engine RAW steps wait on a monotonically counted
semaphore.
"""
from contextlib import ExitStack

import numpy as np
import ml_dtypes

import concourse.bass as bass
from concourse import mybir
from concourse.bass_utils import run_bass_kernel_spmd

F32 = mybir.dt.float32
I32 = mybir.dt.int32
BF16 = mybir.dt.bfloat16
FP8 = mybir.dt.float8e4

P = 128
N_CORES = 8
B = 32768
B_LOC = B // N_CORES          # 4096 rows per core
ROW = 4 * 256                 # 1024 per row per tensor
NG = 512                      # batch rows per matmul group (one PSUM bank)
G = B_LOC // NG               # 8 groups
N_TILES = B_LOC // P          # 32 tiles of 128 rows
NCH = 16                      # K-chunks: 8 slabs (a0..a3,b0..b3) x 2 halves

NBUF = 4                      # input group-buffer slots
OBUF = 4                      # output group-buffer slots
NSUB = 4                      # input sub-DMAs per group
INFLIGHT = 2                  # concurrent group loads
WARMUP_MM = 0                 # dummy matmuls to ramp the PE clock

DVE_OPS = 6                   # s_dve increments per tile (chain ops)
GP_BYTES = ()                 # (Pool has no compare ops; GPSIMD issues stores)
ACT_BYTES = (1, 5)            # encode bytes on ScalarE (square+relu)
PERM = [0, 4, 1, 5, 2, 6, 3, 7]  # output byte e -> idxf column


def _act_bytes(q):
    """ScalarE-encoded bytes for group q."""
    return ACT_BYTES


def _dve_bytes(q):
    return tuple(e for e in range(8)
                 if e not in GP_BYTES and e not in _act_bytes(q))


# cumulative ScalarE relu count through tile t (2 per ACT-encoded tile)
_CUM_AC = []
_c = 0
for _t in range(B_LOC // P):
    _c += len(_act_bytes(_t // 4))
    _CUM_AC.append(_c)


def _op1_count(t):
    """s_dve value once tile t's pt->iv copy has retired (quad interleave)."""
    return 4 * DVE_OPS * (t // 4) + 1 + (t % 4)


def _chain_count(t):
    """s_dve value once tile t's full chain (incl. idxf) has retired."""
    return 4 * DVE_OPS * (t // 4) + 4 * (DVE_OPS - 1) + 1 + (t % 4)


def _build_nc() -> bass.Bass:
    nc = bass.Bass(trn_type="TRN2")
    ab_d = nc.dram_tensor("abt", [G, P, NCH * NG], FP8, kind="ExternalInput")
    tabw_d = nc.dram_tensor("tabw", [P, NCH * 6], BF16, kind="ExternalInput")
    tabio_d = nc.dram_tensor("tabio", [P, 256], BF16, kind="ExternalInput")
    tabid_d = nc.dram_tensor("tabid", [6, 6], F32, kind="ExternalInput")
    out_d = nc.dram_tensor("out", [2, B_LOC, ROW], BF16, kind="ExternalOutput")

    with ExitStack() as ctx:
        sb = lambda name, shape, dt: ctx.enter_context(
            nc.sbuf_tensor(name, shape, dt))
        tabw_t = sb("tabw_t", [P, NCH * 6], BF16)
        tabio_t = sb("tabio_t", [P, 256], BF16)
        tabid_t = sb("tabid_t", [6, 6], F32)
        in_t = [sb(f"in_t{k}", [P, NCH * NG], FP8) for k in range(NBUF)]
        sval = [sb(f"sval{k}", [6, NG], F32) for k in range(2)]
        og = [sb(f"og{k}", [P, 4 * 2 * ROW], BF16) for k in range(OBUF)]
        actsc = sb("actsc", [P, 1], F32)
        # parity-double-buffered per-tile temporaries
        iv = [sb(f"iv_{p}", [P, 8], I32) for p in range(4)]
        idx8 = [sb(f"idx8_{p}", [P, 8], I32) for p in range(4)]
        idxf = [sb(f"idxf_{p}", [P, 8], F32) for p in range(4)]


        pv = [ctx.enter_context(nc.psum_tensor(f"pv{k}", [6, NG], F32))
              for k in range(2)]
        tmpa = [[ctx.enter_context(
            nc.psum_tensor(f"tmpa_{p}_{i}", [P, 256], F32)) if p < 0
            else sb(f"tmpa_{p}_{i}", [P, 256], F32)
            for i in range(max(1, len(ACT_BYTES)))] for p in range(4)]
        pt = [ctx.enter_context(nc.psum_tensor(f"pt{k}", [P, 24], F32))
              for k in range(2)]

        s_tabw = ctx.enter_context(nc.semaphore("s_tabw"))
        s_tabid = ctx.enter_context(nc.semaphore("s_tabid"))
        s_tabio = ctx.enter_context(nc.semaphore("s_tabio"))
        s_sub = [[ctx.enter_context(nc.semaphore(f"s_sub{j}_{u}"))
                  for u in range(NSUB)] for j in range(NBUF)]
        s_store = [ctx.enter_context(nc.semaphore(f"s_store{j}"))
                   for j in range(OBUF)]
        s_mm = ctx.enter_context(nc.semaphore("s_mm"))      # matmul groups
        s_sv = ctx.enter_context(nc.semaphore("s_sv"))      # psum->sbuf copies
        s_T = ctx.enter_context(nc.semaphore("s_T"))        # transposes done
        s_comp = ctx.enter_context(nc.semaphore("s_comp"))  # DVE-encoded tiles
        s_dve = ctx.enter_context(nc.semaphore("s_dve"))    # chain ops done
        s_ac = ctx.enter_context(nc.semaphore("s_ac"))      # ACT-encoded tiles
        s_acq = ctx.enter_context(nc.semaphore("s_acq"))    # ACT square ops

        block = ctx.enter_context(nc.Block())

        @block.sync
        def _(sync: bass.BassEngine):
            CW = NCH * NG // NSUB   # columns per sub-DMA

            def load_group(g):
                j = g % NBUF
                for u in range(NSUB):
                    sync.dma_start(
                        out=in_t[j][:, CW * u:CW * (u + 1)],
                        in_=ab_d[g, :, CW * u:CW * (u + 1)],
                    ).then_inc(s_sub[j][u], 16)

            sync.dma_start(out=tabw_t[:], in_=tabw_d[:]).then_inc(s_tabw, 16)
            load_group(0)
            sync.dma_start(out=tabid_t[:], in_=tabid_d[:]).then_inc(
                s_tabid, 16)
            sync.dma_start(out=tabio_t[:], in_=tabio_d[:]).then_inc(
                s_tabio, 16)
            for g in range(1, G):
                if g == 1:
                    # give group 0 the full bandwidth (critical path)
                    for u in range(NSUB):
                        sync.wait_ge(s_sub[0][u], 16)
                if g >= INFLIGHT:
                    # bounded prefetch: group g-INFLIGHT fully landed first
                    gp_ = g - INFLIGHT
                    for u in range(NSUB):
                        sync.wait_ge(s_sub[gp_ % NBUF][u],
                                     16 * (gp_ // NBUF + 1))
                if g >= NBUF:
                    # slot reuse: matmuls of group g-NBUF consumed it
                    sync.wait_ge(s_mm, g - NBUF + 1)
                load_group(g)

        @block.tensor
        def _(tensor: bass.BassEngine):
            CS = NCH // NSUB
            tensor.wait_ge(s_tabw, 16)
            # clock-ramp warmup while the first input group is in flight
            warm_rhs = tabw_t[:, None, :].to_broadcast((P, 5, NCH * 6))
            for _w in range(WARMUP_MM):
                tensor.matmul(out=pv[1][:, 0:5 * NCH * 6], lhsT=tabw_t[:, 0:6],
                              rhs=warm_rhs, start=True, stop=True)
            for g in range(G + 1):
                def transposes(q):
                    if q == 0:
                        tensor.wait_ge(s_tabid, 16)
                    tensor.wait_ge(s_sv, q + 1)
                    if q >= 2:
                        # pt[q%2] freed once the pt->iv copy of the last
                        # tile of group q-2 retired
                        tensor.wait_ge(s_dve, _op1_count(4 * (q - 2) + 3))
                    for k in range(4):
                        tensor.transpose(
                            out=pt[q % 2][:, 6 * k:6 * (k + 1)],
                            in_=sval[q % 2][:, P * k:P * (k + 1)],
                            identity=tabid_t[:],
                        ).then_inc(s_T, 1)

                # group 0's transposes go before group 1's matmuls so the
                # DVE starts early; later groups keep matmuls first so a
                # late sval copy never stalls the PE pipeline
                if g - 1 == 0:
                    transposes(0)
                if g < G:
                    j = g % NBUF
                    if g >= 2:
                        # pv[g%2] freed once ScalarE copied group g-2
                        tensor.wait_ge(s_sv, g - 1)
                    for c in range(NCH):
                        if c % CS == 0:
                            tensor.wait_ge(s_sub[j][c // CS],
                                           16 * (g // NBUF + 1))
                        ins = tensor.matmul(
                            out=pv[g % 2][:, :],
                            lhsT=tabw_t[:, 6 * c:6 * (c + 1)],
                            rhs=in_t[j][:, NG * c:NG * (c + 1)],
                            start=(c == 0),
                            stop=(c == NCH - 1),
                        )
                        if c == NCH - 1:
                            ins.then_inc(s_mm, 1)
                if g - 1 >= 1:
                    transposes(g - 1)

        @block.scalar
        def _(scalar: bass.BassEngine):
            acq = 0
            # hoist the implicit ACT_TABLE_LOAD off the critical path: the
            # first LUT activation triggers it, so run a dummy early
            scalar.wait_ge(s_tabio, 16)
            scalar.activation(
                out=actsc[:], in_=tabio_t[:, 0:1],
                func=mybir.ActivationFunctionType.Square)
            for g in range(G + 1):
                if g < G:
                    scalar.wait_ge(s_mm, g + 1)
                    if g >= 2:
                        # sval[g%2] freed once transposes of group g-2 done
                        scalar.wait_ge(s_T, 4 * (g - 1))
                    scalar.activation(
                        out=sval[g % 2][:, :], in_=pv[g % 2][:, :],
                        func=mybir.ActivationFunctionType.Copy,
                    ).then_inc(s_sv, 1)
                # ScalarE-encoded bytes for the tiles of group g-1, two
                # tiles interleaved (ACT ops need sems for same-engine RAW)
                qe = g - 1
                if 0 <= qe < G and _act_bytes(qe):
                    joq = qe % OBUF
                    if qe >= OBUF:
                        scalar.wait_ge(s_store[joq], 128 * (qe // OBUF))
                    if qe >= 1:
                        # tmpa WAR: previous group's Relus retired
                        scalar.wait_ge(s_ac, _CUM_AC[4 * (qe - 1) + 3])
                    scalar.wait_ge(s_dve, _chain_count(4 * qe + 3))
                    for pr in range(4):
                        for i, e in enumerate(_act_bytes(qe)):
                            scalar.activation(
                                out=tmpa[pr][i][:], in_=tabio_t[:],
                                func=mybir.ActivationFunctionType.Square,
                                bias=idxf[pr][:, PERM[e]:PERM[e] + 1],
                                scale=-1.0,
                            ).then_inc(s_acq, 1)
                            acq += 1
                    scalar.wait_ge(s_acq, acq)
                    for pr in range(4):
                        for i, e in enumerate(_act_bytes(qe)):
                            scalar.activation(
                                out=og[joq][:, 2048 * pr + 256 * e:
                                            2048 * pr + 256 * (e + 1)],
                                in_=tmpa[pr][i][:],
                                func=mybir.ActivationFunctionType.Relu,
                                bias=1.0, scale=-1.0,
                            ).then_inc(s_ac, 1)

        @block.vector
        def _(vector: bass.BassEngine):
            n = 0  # statically tracked s_dve count

            def chain_op(ins):
                nonlocal n
                ins.then_inc(s_dve, 1)
                n += 1

            PRS = (0, 1, 2, 3)
            for q in range(G):
                jo = q % OBUF
                if q == 0:
                    vector.wait_ge(s_tabio, 16)
                vector.wait_ge(s_T, 4 * (q + 1))
                if q >= 1 and _act_bytes(q - 1):
                    # idxf reuse: ScalarE read group q-1 (squares done)
                    vector.wait_ge(s_acq, _CUM_AC[4 * (q - 1) + 3])
                if q >= OBUF:
                    vector.wait_ge(s_store[jo], 128 * (q // OBUF))
                # interleaved chains: each wait's producers are >=4 ops back
                # iv = [a_lo a_hi b_lo b_hi s_lo_raw s_hi | x_lo x_hi]
                for pr in PRS:
                    chain_op(vector.tensor_copy(
                        iv[pr][:, 0:6], pt[q % 2][:, 6 * pr:6 * pr + 6]))
                vector.wait_ge(s_dve, n)
                for pr in PRS:
                    chain_op(vector.tensor_tensor(
                        out=iv[pr][:, 6:8], in0=iv[pr][:, 0:2],
                        in1=iv[pr][:, 2:4], op=mybir.AluOpType.bitwise_xor))
                    # fold the 2^16 carry into s_hi (s_lo_raw keeps bit 16;
                    # the &255 byte masks strip it later)
                    chain_op(vector.scalar_tensor_tensor(
                        out=iv[pr][:, 5:6], in0=iv[pr][:, 4:5], scalar=65536,
                        in1=iv[pr][:, 5:6],
                        op0=mybir.AluOpType.is_ge, op1=mybir.AluOpType.add))
                vector.wait_ge(s_dve, n)
                for pr in PRS:
                    # byte extract (fused shift+mask); idx8 holds the bytes
                    # in [s0 s2 x0 x2 | s1 s3 x1 x3] order
                    chain_op(vector.tensor_scalar(
                        out=idx8[pr][:, 0:4], in0=iv[pr][:, 4:8], scalar1=255,
                        scalar2=None, op0=mybir.AluOpType.bitwise_and))
                    chain_op(vector.tensor_scalar(
                        out=idx8[pr][:, 4:8], in0=iv[pr][:, 4:8], scalar1=8,
                        scalar2=255,
                        op0=mybir.AluOpType.logical_shift_right,
                        op1=mybir.AluOpType.bitwise_and))
                vector.wait_ge(s_dve, n)
                for pr in PRS:
                    chain_op(vector.tensor_copy(idxf[pr][:], idx8[pr][:]))
                vector.wait_ge(s_dve, n)
                # encode: single-src is_equal against the iota table, one op
                # per output byte, per-partition scalar = that byte's value
                dbytes = _dve_bytes(q)
                for pr in PRS:
                    for i, e in enumerate(dbytes):
                        ins = vector.tensor_scalar(
                            out=og[jo][:, 2048 * pr + 256 * e:
                                       2048 * pr + 256 * (e + 1)],
                            in0=tabio_t[:],
                            scalar1=idxf[pr][:, PERM[e]:PERM[e] + 1],
                            scalar2=None,
                            op0=mybir.AluOpType.is_equal,
                        )
                        if i == len(dbytes) - 1:
                            ins.then_inc(s_comp, 1)

        @block.gpsimd
        def _(gp: bass.BassEngine):
            for t in range(N_TILES):
                q = t // 4
                k = t % 4
                jo = q % OBUF
                r0 = t * P
                gp.wait_ge(s_comp, t + 1)
                if _act_bytes(q):
                    gp.wait_ge(s_ac, _CUM_AC[t])
                gp.dma_start(
                    out=out_d[0, r0:r0 + P, :],
                    in_=og[jo][:, 2048 * k:2048 * k + ROW],
                ).then_inc(s_store[jo], 16)
                gp.dma_start(
                    out=out_d[1, r0:r0 + P, :],
                    in_=og[jo][:, 2048 * k + ROW:2048 * k + 2 * ROW],
                ).then_inc(s_store[jo], 16)

    return nc


def _make_tables():
    pos = np.arange(P, dtype=np.float64)
    w = np.zeros((NCH, P, 6), np.float64)
    for s in range(8):
        col = s // 2 if s < 4 else 2 + (s - 4) // 2
        scol = 4 + (s // 2) % 2
        mul = 1.0 if (s % 2 == 0) else 256.0
        for h in range(2):
            c = 2 * s + h
            v = (pos + 128.0 * h) * mul
            w[c, :, col] = v
            w[c, :, scol] = v
    tabw = w.transpose(1, 0, 2).reshape(P, NCH * 6).astype(ml_dtypes.bfloat16)
    tabio = np.tile(np.arange(256).astype(ml_dtypes.bfloat16)[None, :],
                    (P, 1))
    tabid = np.eye(6, dtype=np.float32)
    return tabw, tabio, tabid


def _pack_core(abt, lo):
    """[NCH, P, B] fp8 slab-chunks -> core block [G, P, NCH*NG]."""
    blk = abt[:, :, lo:lo + B_LOC].reshape(NCH, P, G, NG)
    return np.ascontiguousarray(
        blk.transpose(2, 1, 0, 3).reshape(G, P, NCH * NG))


_NC_CACHE = {}


def _get_nc(variant: str = "main"):
    if variant not in _NC_CACHE:
        _NC_CACHE[variant] = _build_nc()
    return _NC_CACHE[variant]


def _run(a: np.ndarray, b: np.ndarray, **spmd_kwargs):
    assert a.shape == (B, 4, 256) and b.shape == (B, 4, 256)
    a_t = np.ascontiguousarray(
        np.asarray(a, np.float32).reshape(B, 4, 256).transpose(1, 2, 0)
    ).astype(ml_dtypes.float8_e4m3)
    b_t = np.ascontiguousarray(
        np.asarray(b, np.float32).reshape(B, 4, 256).transpose(1, 2, 0)
    ).astype(ml_dtypes.float8_e4m3)
    abt = np.concatenate([a_t.reshape(NCH // 2, P, B),
                          b_t.reshape(NCH // 2, P, B)], axis=0)
    tabw, tabio, tabid = _make_tables()
    in_maps = [
        {
            "abt": _pack_core(abt, i * B_LOC),
            "tabw": tabw,
            "tabio": tabio,
            "tabid": tabid,
        }
        for i in range(N_CORES)
    ]
    nc = _get_nc()
    kr = run_bass_kernel_spmd(nc, in_maps, list(range(N_CORES)), **spmd_kwargs)
    shards = [kr.results[i]["out"] for i in range(N_CORES)]
    out = np.concatenate(shards, axis=1).astype(np.float32)
    return out.reshape(2, B, 4, 256), kr


def kernel(a: np.ndarray, b: np.ndarray) -> np.ndarray:
    out, _ = _run(a, b)
    return out


def run_sim():
    """CoreSim one core vs numpy oracle (invoked by test.py --sim)."""
    from concourse.bass_interp import CoreSim

    rng = np.random.default_rng(1)
    Bl = B_LOC
    ai = rng.integers(0, 256, (Bl, 4))
    bi = rng.integers(0, 256, (Bl, 4))
    ai[0] = [255] * 4
    bi[0] = [255] * 4
    ai[1] = [255, 255, 255, 255]
    bi[1] = [1, 0, 0, 0]
    a = np.zeros((Bl, 4, 256), np.float32)
    b = np.zeros((Bl, 4, 256), np.float32)
    r = np.arange(Bl)[:, None]
    j = np.arange(4)[None, :]
    a[r, j, ai] = 1.0
    b[r, j, bi] = 1.0

    a_t = np.ascontiguousarray(a.transpose(1, 2, 0)).astype(
        ml_dtypes.float8_e4m3)
    b_t = np.ascontiguousarray(b.transpose(1, 2, 0)).astype(
        ml_dtypes.float8_e4m3)
    abt = np.concatenate([a_t.reshape(NCH // 2, P, Bl),
                          b_t.reshape(NCH // 2, P, Bl)], axis=0)
    tabw, tabio, tabid = _make_tables()

    nc = _get_nc()
    sim = CoreSim(nc)
    sim.tensor("abt")[:] = _pack_core(abt, 0)
    sim.tensor("tabw")[:] = tabw
    sim.tensor("tabio")[:] = tabio
    sim.tensor("tabid")[:] = tabid
    sim.simulate()
    out = np.array(sim.tensor("out")).astype(np.float32).reshape(2, Bl, 4, 256)

    # numpy oracle
    pw = (256 ** np.arange(4)).astype(np.int64)
    a32 = (ai * pw).sum(-1)
    b32 = (bi * pw).sum(-1)
    s32 = (a32 + b32) % (2 ** 32)
    x32 = a32 ^ b32
    sb_ = np.stack([(s32 >> (8 * i)) & 255 for i in range(4)], -1)
    xb_ = np.stack([(x32 >> (8 * i)) & 255 for i in range(4)], -1)
    exp = np.zeros((2, Bl, 4, 256), np.float32)
    exp[0, r, j, sb_] = 1.0
    exp[1, r, j, xb_] = 1.0
    err = np.abs(out - exp).max()
    print(f"SIM max abs err: {err}")
    assert err == 0.0, "sim mismatch"
    print("SIM PASS")
